# revision 19
# baseline (speedup 1.0000x reference)
"""Causal self-attention (transformer block) on 8 trn2 NeuronCores.

Data-parallel over batch: core i processes batch element i (B=8).
Per-core dataflow (T=1024, C=768, H=12 heads, hd=64), all matmul
operands fp16 with fp32 PSUM accumulation:

  x [T,C] --PE transpose--> xT [C,T]            (feature-major)
  qkT[m]  = W_attn[:,m].T @ xT + b  [1536,T]    (feature-major q,k)
  v[t]    = xT[t].T @ W_attn[:,v] + b [T,768]   (row-major, +ones col)
  S^T[j,i] = sum_d kT[d,j] qT[d,i]              (scores transposed)
  E = exp(S^T*scale + causal_mask)              (masked -> exact 0)
  psY = sum_j [v_j | 1].T @ E_j                 (row 64 = softmax denom)
  yT = psY[0:64] * bcast(1/psY[64])             (feature-major y)
  out[t] = yT[:,t].T @ W_proj + b               (row-major, DMA out)
"""
import numpy as np

import concourse.bass as bass
import concourse.tile as tile
from concourse import bacc, mybir
from concourse.bass_utils import run_bass_kernel_spmd
from concourse.masks import make_identity

f32 = mybir.dt.float32
f16 = mybir.dt.float16
Exp = mybir.ActivationFunctionType.Exp

B = 8
T = 1024
C = 768
H = 12
HD = 64
SCALE = HD ** -0.5
NEG = -1e9
KC = C // 128        # 6 feature chunks
MT = T // 128        # 8 token tiles
GW = 512             # Tq group width
NG = T // GW         # 4 groups
VW = H * (HD + 1)    # v tile width incl. ones column (780)


def build_nc():
    nc = bacc.Bacc(None)
    x = nc.dram_tensor("x", [T, C], f32, kind="ExternalInput")
    W_attn = nc.dram_tensor("W_attn", [C, 3 * C], f32, kind="ExternalInput")
    b_attn = nc.dram_tensor("b_attn", [3 * C], f32, kind="ExternalInput")
    W_proj = nc.dram_tensor("W_proj", [C, C], f32, kind="ExternalInput")
    b_proj = nc.dram_tensor("b_proj", [C], f32, kind="ExternalInput")
    out = nc.dram_tensor("out", [T, C], f32, kind="ExternalOutput")

    with tile.TileContext(nc) as tc:
        with (
            tc.tile_pool(name="consts", bufs=1) as consts,
            tc.tile_pool(name="stage", bufs=2) as stage,
            tc.tile_pool(name="x16p", bufs=3) as x16p,
            tc.tile_pool(name="wq", bufs=1) as wq,
            tc.tile_pool(name="wp", bufs=1) as wp,
            tc.tile_pool(name="big", bufs=1) as big,
            tc.tile_pool(name="ep", bufs=6) as ep,
            tc.tile_pool(name="small", bufs=4) as small,
            tc.tile_pool(name="outp", bufs=2) as outp,
            tc.tile_pool(name="yup", bufs=6) as yup,
            tc.tile_pool(name="psG", bufs=2, space="PSUM") as psG,
            tc.tile_pool(name="psA", bufs=3, space="PSUM") as psA,
            tc.tile_pool(name="psY", bufs=3, space="PSUM") as psY,
        ):
            # ---- constants ----
            ident = consts.tile([128, 128], f16, tag="ident")
            make_identity(nc, ident[:, :])

            # additive causal mask for the S^T diagonal 128x128 block:
            # keep where p<=f, NEG elsewhere.
            maskA = consts.tile([128, 128], f32, tag="maskA")
            nc.gpsimd.memset(maskA[:, :], 0.0)
            nc.gpsimd.affine_select(
                out=maskA[:, :], in_=maskA[:, :],
                compare_op=mybir.AluOpType.is_ge, fill=NEG,
                base=0, pattern=[[1, 128]], channel_multiplier=-1,
            )
            ones16 = consts.tile([1, 512], f16, tag="ones16")
            nc.vector.memset(ones16[:, :], 1.0)
            # all-ones rows at every 32-aligned partition, for matmuls whose
            # rhs lives at a nonzero base partition (base partitions must match)
            ones65 = consts.tile([65, 64], f16, tag="ones65")
            nc.vector.memset(ones65[:, :], 1.0)

            b32 = stage.tile([1, 3 * C], f32, tag="b32")
            nc.sync.dma_start(out=b32[:, :], in_=b_attn.ap().rearrange("(a d) -> a d", a=1))
            b16 = consts.tile([1, 3 * C], f16, tag="b16")
            nc.vector.tensor_copy(b16[:, :], b32[:, :])

            bp32 = stage.tile([1, C], f32, tag="bp32")
            nc.sync.dma_start(out=bp32[:, :], in_=b_proj.ap().rearrange("(a d) -> a d", a=1))
            bp16 = consts.tile([1, C], f16, tag="bp16")
            nc.vector.tensor_copy(bp16[:, :], bp32[:, :])

            # ---- weights: load + fp16 convert ----
            W16 = []
            for k in range(KC):
                w32 = stage.tile([128, 3 * C], f32, tag="w32")
                nc.sync.dma_start(out=w32[:, :], in_=W_attn[k * 128:(k + 1) * 128, :])
                wt = wq.tile([128, 3 * C], f16, tag=f"W16_{k}", name=f"W16_{k}")
                nc.scalar.copy(wt[:, :], w32[:, :])
                W16.append(wt)
            Wp16 = []
            for k in range(KC):
                w32 = stage.tile([128, C], f32, tag="wp32")
                nc.sync.dma_start(out=w32[:, :], in_=W_proj[k * 128:(k + 1) * 128, :])
                wt = wp.tile([128, C], f16, tag=f"Wp16_{k}", name=f"Wp16_{k}")
                nc.vector.tensor_copy(wt[:, :], w32[:, :])
                Wp16.append(wt)

            # ---- x load, fp16 convert, transpose to xT [C, T] ----
            xT = [big.tile([128, T], f16, tag=f"xT_{c}", name=f"xT_{c}") for c in range(KC)]
            for t in range(MT):
                x32 = stage.tile([128, C], f32, tag="x32")
                nc.sync.dma_start(out=x32[:, :], in_=x[t * 128:(t + 1) * 128, :])
                x16 = x16p.tile([128, C], f16, tag="x16")
                nc.vector.tensor_copy(x16[:, :], x32[:, :])
                for c in range(KC):
                    pst = psG.tile([128, 128], f16, tag="g")
                    nc.tensor.transpose(pst[:, :], x16[:, c * 128:(c + 1) * 128], ident[:, :])
                    nc.vector.tensor_copy(xT[c][:, t * 128:(t + 1) * 128], pst[:, :])

            # ---- qk^T GEMM: qkT[m] [128, T] f16, m=0..11 covers features 0..1535
            # token-column halves (n) are emitted separately: n=0 upfront,
            # n=1 as PE filler inside attention group 0.
            qkT = [big.tile([128, T], f16, tag=f"qkT_{m}", name=f"qkT_{m}") for m in range(12)]

            def emit_qk(m, n):
                ps = psG.tile([128, 512], f32, tag="g", name="qk_ps")
                nc.tensor.matmul(
                    ps[:, :], b16[0:1, m * 128:(m + 1) * 128],
                    ones16[0:1, 0:512], start=True, stop=False)
                for k in range(KC):
                    nc.tensor.matmul(
                        ps[:, :], W16[k][:, m * 128:(m + 1) * 128],
                        xT[k][:, n * 512:(n + 1) * 512],
                        start=False, stop=(k == KC - 1))
                nc.scalar.copy(qkT[m][:, n * 512:(n + 1) * 512], ps[:, :])

            for m in range(12):
                emit_qk(m, 0)
                emit_qk(m, 1)

            # ---- v rows: v_sb[t] [128, 780] f16 (64 cols + ones col per head)
            # t=0..3 upfront; t=4..7 as PE filler inside attention group 0.
            v_sb = [big.tile([128, VW], f16, tag=f"v_{t}", name=f"v_{t}") for t in range(MT)]

            def emit_v(t):
                vht = v_sb[t][:, :].rearrange("p (h s) -> p h s", s=HD + 1)
                nc.vector.memset(vht[:, :, HD:HD + 1], 1.0)
                vchunks = ((0, 512), (512, 256))
                pss = [psG.tile([128, 512], f32, tag="g", name=f"v_ps{n}")
                       for n in range(2)]
                for n, (c0, w) in enumerate(vchunks):
                    nc.tensor.matmul(
                        pss[n][:, 0:w], ones16[0:1, 0:128],
                        b16[0:1, 2 * C + c0:2 * C + c0 + w], start=True, stop=False)
                for k in range(KC):
                    for n, (c0, w) in enumerate(vchunks):
                        nc.tensor.matmul(
                            pss[n][:, 0:w], xT[k][:, t * 128:(t + 1) * 128],
                            W16[k][:, 2 * C + c0:2 * C + c0 + w],
                            start=False, stop=(k == KC - 1))
                for n, (c0, w) in enumerate(vchunks):
                    nh = w // HD
                    h0 = c0 // HD
                    nc.vector.tensor_copy(
                        vht[:, h0:h0 + nh, 0:HD],
                        pss[n][:, 0:w].rearrange("p (h s) -> p h s", s=HD))

            for t in range(MT):
                emit_v(t)

            # ---- attention: S^T -> mask -> exp -> AV (+denom) -> normalize
            yT = [big.tile([128, T], f16, tag=f"yT_{m}", name=f"yT_{m}") for m in range(KC)]
            def emit_proj(t):
                o_sb = outp.tile([128, C], f32, tag="o", name="o_sb")
                ochunks = ((0, 512), (512, 256))
                pss = [psG.tile([128, 512], f32, tag="g", name=f"o_ps{n}")
                       for n in range(2)]
                for n, (c0, w) in enumerate(ochunks):
                    nc.tensor.matmul(
                        pss[n][:, 0:w], ones16[0:1, 0:128],
                        bp16[0:1, c0:c0 + w], start=True, stop=False)
                for k in range(KC):
                    for n, (c0, w) in enumerate(ochunks):
                        nc.tensor.matmul(
                            pss[n][:, 0:w], yT[k][:, t * 128:(t + 1) * 128],
                            Wp16[k][:, c0:c0 + w],
                            start=False, stop=(k == KC - 1))
                for n, (c0, w) in enumerate(ochunks):
                    nc.scalar.copy(o_sb[:, c0:c0 + w], pss[n][:, 0:w])
                nc.sync.dma_start(out=out[t * 128:(t + 1) * 128, :], in_=o_sb[:, :])

            for g in range(NG):
                for hq in range(H // 3):
                    # 3-head triad, interleaved at chunk level so PE always has
                    # independent matmuls while ACT computes exps. Denominator
                    # rows live at 32-aligned partitions (base must be 0/32/64)
                    # so one DVE reciprocal serves all three heads.
                    den4 = small.tile([65, GW], f32, tag="den4",
                                      name=f"den4_{g}_{hq}")
                    nchunks = 4 * g + 4
                    heads = [(hq * 3 + hh, hh) for hh in range(3)]
                    psys = [psY.tile([65, GW], f32, tag="y", name=f"psy{hh}")
                            for hh in range(3)]
                    for j in range(nchunks):
                        Es = []
                        for h, hh in heads:
                            qt, qp = h // 2, (h % 2) * 64
                            pss = psA.tile([128, GW], f32, tag="s", name="pss")
                            nc.tensor.matmul(
                                pss[:, :],
                                qkT[6 + qt][qp:qp + 64, j * 128:(j + 1) * 128],
                                qkT[qt][qp:qp + 64, g * GW:(g + 1) * GW],
                                start=True, stop=True)
                            E = ep.tile([128, GW], f16, tag="e", name="E")
                            cd = j - 4 * g  # diagonal col-block index
                            if cd < 0:
                                nc.scalar.activation(E[:, :], pss[:, :], Exp, scale=SCALE)
                            else:
                                c0 = cd * 128
                                if c0 > 0:
                                    nc.gpsimd.memset(E[:, 0:c0], 0.0)
                                nc.vector.tensor_add(
                                    pss[:, c0:c0 + 128], pss[:, c0:c0 + 128], maskA[:, :])
                                nc.scalar.activation(
                                    E[:, c0:GW], pss[:, c0:GW], Exp, scale=SCALE)
                            Es.append(E)
                        for (h, hh), E in zip(heads, Es):
                            nc.tensor.matmul(
                                psys[hh][:, :],
                                v_sb[j][:, h * (HD + 1):(h + 1) * (HD + 1)],
                                E[:, :], start=(j == 0), stop=(j == nchunks - 1))
                    yu_list = []
                    for h, hh in heads:
                        yu = yup.tile([64, GW], f16, tag="yu", name="yu")
                        nc.vector.tensor_copy(yu[:, :], psys[hh][0:64, :])
                        nc.vector.tensor_copy(
                            den4[32 * hh:32 * hh + 1, :], psys[hh][64:65, :])
                        yu_list.append((h, yu))
                    rec4 = small.tile([65, GW], f32, tag="rec4",
                                      name=f"rec4_{g}_{hq}")
                    nc.vector.reciprocal(rec4[:, :], den4[:, :])
                    rec16 = small.tile([65, GW], f16, tag="rec16",
                                       name=f"rec16_{g}_{hq}")
                    nc.vector.tensor_copy(rec16[:, :], rec4[:, :])
                    for hh, (h, yu) in enumerate(yu_list):
                        qt, qp = h // 2, (h % 2) * 64
                        psb = psA.tile([64, GW], f32, tag="s", name="psb")
                        nc.tensor.matmul(
                            psb[:, :], ones65[32 * hh:32 * hh + 1, :],
                            rec16[32 * hh:32 * hh + 1, :], start=True, stop=True)
                        nc.vector.tensor_mul(
                            yT[qt][qp:qp + 64, g * GW:(g + 1) * GW],
                            yu[:, :], psb[:, :])
                    # stagger the previous group's output projection between
                    # triads to keep PE fed while ACT works on exps
                    if g > 0:
                        emit_proj(4 * (g - 1) + hq)
            for t in range(4 * (NG - 1), 4 * NG):
                emit_proj(t)

    nc.finalize()
    return nc


_CACHE = {}


def _get_nc():
    if "nc" not in _CACHE:
        _CACHE["nc"] = build_nc()
    return _CACHE["nc"]


def run(inputs, trace=False):
    nc = _get_nc()
    x = np.asarray(inputs["x"], dtype=np.float32)
    in_maps = [
        {
            "x": np.ascontiguousarray(x[i]),
            "W_attn": np.asarray(inputs["W_attn"], dtype=np.float32),
            "b_attn": np.asarray(inputs["b_attn"], dtype=np.float32),
            "W_proj": np.asarray(inputs["W_proj"], dtype=np.float32),
            "b_proj": np.asarray(inputs["b_proj"], dtype=np.float32),
        }
        for i in range(B)
    ]
    res = run_bass_kernel_spmd(nc, in_maps, core_ids=list(range(B)), trace=trace)
    y = np.stack([res.results[i]["out"] for i in range(B)], axis=0)
    return y, res


def kernel(**inputs):
    y, _ = run(inputs, trace=False)
    return y


# revision 22
# speedup vs baseline: 1.2329x; 1.2329x over previous
"""Causal self-attention (transformer block) on 8 trn2 NeuronCores.

Data-parallel over batch: core i processes batch element i (B=8).
Per-core dataflow (T=1024, C=768, H=12 heads, hd=64), all matmul
operands fp16 with fp32 PSUM accumulation:

  x [T,C] --PE transpose--> xT [C,T]            (feature-major)
  qkT[m]  = W_attn[:,m].T @ xT + b  [1536,T]    (feature-major q,k)
  v[t]    = xT[t].T @ W_attn[:,v] + b [T,768]   (row-major, +ones col)
  S^T[j,i] = sum_d kT[d,j] qT[d,i]              (scores transposed)
  E = exp(S^T*scale + causal_mask)              (masked -> exact 0)
  psY = sum_j [v_j | 1].T @ E_j                 (row 64 = softmax denom)
  yT = psY[0:64] * bcast(1/psY[64])             (feature-major y)
  out[t] = yT[:,t].T @ W_proj + b               (row-major, DMA out)
"""
import numpy as np

import concourse.bass as bass
import concourse.tile as tile
from concourse import bacc, mybir
from concourse.bass_utils import run_bass_kernel_spmd
from concourse.masks import make_identity

f32 = mybir.dt.float32
f16 = mybir.dt.float16
Exp = mybir.ActivationFunctionType.Exp

B = 8
T = 1024
C = 768
H = 12
HD = 64
SCALE = HD ** -0.5
NEG = -1e9
KC = C // 128        # 6 feature chunks
MT = T // 128        # 8 token tiles
GW = 512             # Tq group width
NG = T // GW         # 4 groups
VW = H * (HD + 1)    # v tile width incl. ones column (780)


def build_nc():
    nc = bacc.Bacc(None)
    x = nc.dram_tensor("x", [T, C], f32, kind="ExternalInput")
    W_attn = nc.dram_tensor("W_attn", [C, 3 * C], f32, kind="ExternalInput")
    b_attn = nc.dram_tensor("b_attn", [3 * C], f32, kind="ExternalInput")
    W_proj = nc.dram_tensor("W_proj", [C, C], f32, kind="ExternalInput")
    b_proj = nc.dram_tensor("b_proj", [C], f32, kind="ExternalInput")
    out = nc.dram_tensor("out", [T, C], f32, kind="ExternalOutput")

    with tile.TileContext(nc) as tc:
        with (
            tc.tile_pool(name="consts", bufs=1) as consts,
            tc.tile_pool(name="stage", bufs=2) as stage,
            tc.tile_pool(name="x16p", bufs=3) as x16p,
            tc.tile_pool(name="wq", bufs=1) as wq,
            tc.tile_pool(name="wp", bufs=1) as wp,
            tc.tile_pool(name="big", bufs=1) as big,
            tc.tile_pool(name="ep", bufs=4) as ep,
            tc.tile_pool(name="small", bufs=4) as small,
            tc.tile_pool(name="outp", bufs=2) as outp,
            tc.tile_pool(name="yup", bufs=6) as yup,
            tc.tile_pool(name="psG", bufs=2, space="PSUM") as psG,
            tc.tile_pool(name="psA", bufs=2, space="PSUM") as psA,
            tc.tile_pool(name="psY", bufs=2, space="PSUM") as psY,
        ):
            # ---- constants ----
            ident = consts.tile([128, 128], f16, tag="ident")
            make_identity(nc, ident[:, :])

            # additive causal mask for the S^T diagonal 128x128 block:
            # keep where p<=f, NEG elsewhere.
            maskA = consts.tile([128, 128], f32, tag="maskA")
            nc.gpsimd.memset(maskA[:, :], 0.0)
            nc.gpsimd.affine_select(
                out=maskA[:, :], in_=maskA[:, :],
                compare_op=mybir.AluOpType.is_ge, fill=NEG,
                base=0, pattern=[[1, 128]], channel_multiplier=-1,
            )
            ones16 = consts.tile([1, 512], f16, tag="ones16")
            nc.vector.memset(ones16[:, :], 1.0)
            # all-ones rows at every 32-aligned partition, for matmuls whose
            # rhs lives at a nonzero base partition (base partitions must match)
            ones65 = consts.tile([65, 64], f16, tag="ones65")
            nc.vector.memset(ones65[:, :], 1.0)

            b32 = stage.tile([1, 3 * C], f32, tag="b32")
            nc.sync.dma_start(out=b32[:, :], in_=b_attn.ap().rearrange("(a d) -> a d", a=1))
            b16 = consts.tile([1, 3 * C], f16, tag="b16")
            nc.vector.tensor_copy(b16[:, :], b32[:, :])

            bp32 = stage.tile([1, C], f32, tag="bp32")
            nc.sync.dma_start(out=bp32[:, :], in_=b_proj.ap().rearrange("(a d) -> a d", a=1))
            bp16 = consts.tile([1, C], f16, tag="bp16")
            nc.vector.tensor_copy(bp16[:, :], bp32[:, :])

            # ---- weights: load + fp16 convert ----
            W16 = []
            for k in range(KC):
                w32 = stage.tile([128, 3 * C], f32, tag="w32")
                nc.sync.dma_start(out=w32[:, :], in_=W_attn[k * 128:(k + 1) * 128, :])
                wt = wq.tile([128, 3 * C], f16, tag=f"W16_{k}", name=f"W16_{k}")
                nc.scalar.copy(wt[:, :], w32[:, :])
                W16.append(wt)
            Wp16 = []
            for k in range(KC):
                w32 = stage.tile([128, C], f32, tag="wp32")
                nc.sync.dma_start(out=w32[:, :], in_=W_proj[k * 128:(k + 1) * 128, :])
                wt = wp.tile([128, C], f16, tag=f"Wp16_{k}", name=f"Wp16_{k}")
                nc.vector.tensor_copy(wt[:, :], w32[:, :])
                Wp16.append(wt)

            # ---- x load, fp16 convert, transpose to xT [C, T] ----
            xT = [big.tile([128, T], f16, tag=f"xT_{c}", name=f"xT_{c}") for c in range(KC)]
            for t in range(MT):
                x32 = stage.tile([128, C], f32, tag="x32")
                nc.sync.dma_start(out=x32[:, :], in_=x[t * 128:(t + 1) * 128, :])
                x16 = x16p.tile([128, C], f16, tag="x16")
                nc.vector.tensor_copy(x16[:, :], x32[:, :])
                for c in range(KC):
                    pst = psG.tile([128, 128], f16, tag="g")
                    nc.tensor.transpose(pst[:, :], x16[:, c * 128:(c + 1) * 128], ident[:, :])
                    nc.vector.tensor_copy(xT[c][:, t * 128:(t + 1) * 128], pst[:, :])

            # ---- qk^T GEMM: qkT[m] [128, T] f16, m=0..11 covers features 0..1535
            # token-column halves (n) are emitted separately: n=0 upfront,
            # n=1 as PE filler inside attention group 0.
            qkT = [big.tile([128, T], f16, tag=f"qkT_{m}", name=f"qkT_{m}") for m in range(12)]

            def emit_qk(m, n):
                ps = psG.tile([128, 512], f32, tag="g", name="qk_ps")
                nc.tensor.matmul(
                    ps[:, :], b16[0:1, m * 128:(m + 1) * 128],
                    ones16[0:1, 0:512], start=True, stop=False)
                for k in range(KC):
                    nc.tensor.matmul(
                        ps[:, :], W16[k][:, m * 128:(m + 1) * 128],
                        xT[k][:, n * 512:(n + 1) * 512],
                        start=False, stop=(k == KC - 1))
                nc.scalar.copy(qkT[m][:, n * 512:(n + 1) * 512], ps[:, :])

            for m in range(12):
                emit_qk(m, 0)
                emit_qk(m, 1)

            # ---- v rows: v_sb[t] [128, 780] f16 (64 cols + ones col per head)
            # t=0..3 upfront; t=4..7 as PE filler inside attention group 0.
            v_sb = [big.tile([128, VW], f16, tag=f"v_{t}", name=f"v_{t}") for t in range(MT)]

            def emit_v(t):
                vht = v_sb[t][:, :].rearrange("p (h s) -> p h s", s=HD + 1)
                nc.vector.memset(vht[:, :, HD:HD + 1], 1.0)
                vchunks = ((0, 512), (512, 256))
                pss = [psG.tile([128, 512], f32, tag="g", name=f"v_ps{n}")
                       for n in range(2)]
                for n, (c0, w) in enumerate(vchunks):
                    nc.tensor.matmul(
                        pss[n][:, 0:w], ones16[0:1, 0:128],
                        b16[0:1, 2 * C + c0:2 * C + c0 + w], start=True, stop=False)
                for k in range(KC):
                    for n, (c0, w) in enumerate(vchunks):
                        nc.tensor.matmul(
                            pss[n][:, 0:w], xT[k][:, t * 128:(t + 1) * 128],
                            W16[k][:, 2 * C + c0:2 * C + c0 + w],
                            start=False, stop=(k == KC - 1))
                for n, (c0, w) in enumerate(vchunks):
                    nh = w // HD
                    h0 = c0 // HD
                    nc.vector.tensor_copy(
                        vht[:, h0:h0 + nh, 0:HD],
                        pss[n][:, 0:w].rearrange("p (h s) -> p h s", s=HD))

            for t in range(MT):
                emit_v(t)

            # ---- attention: S^T -> mask -> exp -> AV (+denom) -> normalize
            yT = [big.tile([128, T], f16, tag=f"yT_{m}", name=f"yT_{m}") for m in range(KC)]
            def emit_proj(t):
                o_sb = outp.tile([128, C], f32, tag="o", name="o_sb")
                ochunks = ((0, 512), (512, 256))
                pss = [psG.tile([128, 512], f32, tag="g", name=f"o_ps{n}")
                       for n in range(2)]
                for n, (c0, w) in enumerate(ochunks):
                    nc.tensor.matmul(
                        pss[n][:, 0:w], ones16[0:1, 0:128],
                        bp16[0:1, c0:c0 + w], start=True, stop=False)
                for k in range(KC):
                    for n, (c0, w) in enumerate(ochunks):
                        nc.tensor.matmul(
                            pss[n][:, 0:w], yT[k][:, t * 128:(t + 1) * 128],
                            Wp16[k][:, c0:c0 + w],
                            start=False, stop=(k == KC - 1))
                for n, (c0, w) in enumerate(ochunks):
                    nc.scalar.copy(o_sb[:, c0:c0 + w], pss[n][:, 0:w])
                nc.sync.dma_start(out=out[t * 128:(t + 1) * 128, :], in_=o_sb[:, :])

            for g in range(NG):
                yus = {}
                den4s = [small.tile([65, GW], f32, tag="den4",
                                    name=f"den4_{g}_{q}") for q in range(4)]
                # head pairs (2p, 2p+1) share one qkT tile at partitions 0/64;
                # their S chunks land in one 2-bank PSUM tile so a single exp
                # instruction serves both heads.
                for pr in range(H // 2):
                    hA, hB = 2 * pr, 2 * pr + 1
                    qt = pr
                    nchunks = 4 * g + 4
                    psyA = psY.tile([65, GW], f32, tag="y", name="psyA")
                    psyB = psY.tile([65, GW], f32, tag="y", name="psyB")
                    for j in range(nchunks):
                        psS = psA.tile([128, 2, GW], f32, tag="s", name="psS")
                        nc.tensor.matmul(
                            psS[:, 0, :],
                            qkT[6 + qt][0:64, j * 128:(j + 1) * 128],
                            qkT[qt][0:64, g * GW:(g + 1) * GW],
                            start=True, stop=True)
                        nc.tensor.matmul(
                            psS[:, 1, :],
                            qkT[6 + qt][64:128, j * 128:(j + 1) * 128],
                            qkT[qt][64:128, g * GW:(g + 1) * GW],
                            start=True, stop=True)
                        E2 = ep.tile([128, 2, GW], f16, tag="e", name="E2")
                        cd = j - 4 * g  # diagonal col-block index
                        if cd < 0:
                            nc.scalar.activation(
                                E2[:, :, :], psS[:, :, :], Exp, scale=SCALE)
                        else:
                            c0 = cd * 128
                            if c0 > 0:
                                nc.gpsimd.memset(E2[:, 0, 0:c0], 0.0)
                                nc.gpsimd.memset(E2[:, 1, 0:c0], 0.0)
                            nc.vector.tensor_add(
                                psS[:, 0, c0:c0 + 128], psS[:, 0, c0:c0 + 128],
                                maskA[:, :])
                            nc.vector.tensor_add(
                                psS[:, 1, c0:c0 + 128], psS[:, 1, c0:c0 + 128],
                                maskA[:, :])
                            nc.scalar.activation(
                                E2[:, :, c0:GW], psS[:, :, c0:GW], Exp, scale=SCALE)
                        nc.tensor.matmul(
                            psyA[:, :],
                            v_sb[j][:, hA * (HD + 1):(hA + 1) * (HD + 1)],
                            E2[:, 0, :], start=(j == 0), stop=(j == nchunks - 1))
                        nc.tensor.matmul(
                            psyB[:, :],
                            v_sb[j][:, hB * (HD + 1):(hB + 1) * (HD + 1)],
                            E2[:, 1, :], start=(j == 0), stop=(j == nchunks - 1))
                    for h, psy in ((hA, psyA), (hB, psyB)):
                        yu = yup.tile([64, GW], f16, tag="yu", name="yu")
                        nc.vector.tensor_copy(yu[:, :], psy[0:64, :])
                        nc.vector.tensor_copy(
                            den4s[h // 3][32 * (h % 3):32 * (h % 3) + 1, :],
                            psy[64:65, :])
                        yus[h] = yu
                # per-triad reciprocal + normalize, with the previous group's
                # output projection staggered between triads to keep PE fed
                for hq in range(4):
                    rec4 = small.tile([65, GW], f32, tag="rec4",
                                      name=f"rec4_{g}_{hq}")
                    nc.vector.reciprocal(rec4[:, :], den4s[hq][:, :])
                    rec16 = small.tile([65, GW], f16, tag="rec16",
                                       name=f"rec16_{g}_{hq}")
                    nc.vector.tensor_copy(rec16[:, :], rec4[:, :])
                    for hh in range(3):
                        h = hq * 3 + hh
                        qt, qp = h // 2, (h % 2) * 64
                        psb = psG.tile([64, GW], f32, tag="g", name="psb")
                        nc.tensor.matmul(
                            psb[:, :], ones65[32 * hh:32 * hh + 1, :],
                            rec16[32 * hh:32 * hh + 1, :], start=True, stop=True)
                        nc.vector.tensor_mul(
                            yT[qt][qp:qp + 64, g * GW:(g + 1) * GW],
                            yus[h][:, :], psb[:, :])
                    if g > 0:
                        emit_proj(4 * (g - 1) + hq)
            for t in range(4 * (NG - 1), 4 * NG):
                emit_proj(t)

    nc.finalize()
    return nc


_CACHE = {}


def _get_nc():
    if "nc" not in _CACHE:
        _CACHE["nc"] = build_nc()
    return _CACHE["nc"]


def run(inputs, trace=False):
    nc = _get_nc()
    x = np.asarray(inputs["x"], dtype=np.float32)
    in_maps = [
        {
            "x": np.ascontiguousarray(x[i]),
            "W_attn": np.asarray(inputs["W_attn"], dtype=np.float32),
            "b_attn": np.asarray(inputs["b_attn"], dtype=np.float32),
            "W_proj": np.asarray(inputs["W_proj"], dtype=np.float32),
            "b_proj": np.asarray(inputs["b_proj"], dtype=np.float32),
        }
        for i in range(B)
    ]
    res = run_bass_kernel_spmd(nc, in_maps, core_ids=list(range(B)), trace=trace)
    y = np.stack([res.results[i]["out"] for i in range(B)], axis=0)
    return y, res


def kernel(**inputs):
    y, _ = run(inputs, trace=False)
    return y
